# revision 30
# baseline (speedup 1.0000x reference)
"""Multi-head attention Bass/Tile kernel for Trainium2.

Problem: nn_MultiHeadAttention  (B=8, S=1024, D=768, H=12, HD=64)
  q = x_h @ Wq^T + bq ; k,v likewise (per head)
  scores = q @ k^T        (NO pre-softmax scaling)
  attn = softmax(scores, -1) / sqrt(64)
  out = attn @ v, heads concatenated -> [B, S, D]

Sharding: data parallel over batch, one sample per NeuronCore (8 cores).

All big matmuls run as float32r (1 cycle/row when N>=256, vs 4 for fp32).
fp32r ISA restrictions shape the layouts: every fp32r matmul needs M (out
partitions) spanning all 128 PE columns, even moving-dim counts, and
8-byte-aligned contiguous dst.

Per-core dataflow:
  xt[h]   : [65, 1024]  x_b^T per head + ones row (folds biases via K=65)
  wqk[h]  : [65, 128]   [[Wq^T | Wk^T], [bq | bk]]   -> QK^T   [128, 1024]
  wzq[h]  : [65, 128]   [[0 | Wq^T], [0 | bq]]       -> ZQ     [128, 1024]
      (scoresT = qkt_chunk.T @ zq: the zero rows of ZQ annihilate the
       Q-rows of the lhsT chunk, leaving K^T.T @ Q^T = scores transposed.
       K=128 keeps the whole PE array active so the HAM clock gate stays
       at full speed; K=64 matmuls read as half-idle and get throttled.)
  wv[h]   : [65, 66]    [[Wv^T, 0, 0], [bv, 8, 0]]   (66 = even-N pad;
       col 64 gives an 8*ones column -> attn row-sums * sqrt(64) for free)
  V'      : [128, 66] per t-chunk = x_chunk @ wv
  scoresT : [128, 1024] per t-chunk = K^T[:,tc].T @ Q^T
  attnT   : exp(scoresT)  (unnormalized; no max-subtraction needed in fp32)
  OT'     : [128, 1024]  = sum_tc  attnT[tc]^T.T @ V'[tc]... realized as
            lhsT=vp[66tc : 66tc+128] (M=128 incl. garbage cols), rhs=attnT
            -> rows 0-63 = out^T, row 64 = 8*rowsum, rows 65+ garbage
  OTT     : PE-transpose back to [s, e] chunks; col 64 = 8*rowsum
  y       : OTT[:, :64] * (1 / OTT[:, 64])
"""

import os
import sys

for _p in (
    "/opt/trn_rl_repo",
    "/root/.axon_site",
    "/root/.axon_site/_ro/trn_rl_repo",
    "/root/.axon_site/_ro/pypackages",
):
    if os.path.isdir(_p) and _p not in sys.path:
        sys.path.append(_p)

import numpy as np

import concourse.bacc as bacc
import concourse.bass as bass
import concourse.tile as tile
from concourse import mybir

B, S, D, H, HD = 8, 1024, 768, 12, 64
K1 = HD + 1  # 65: contraction dim with ones row for bias folding
VW = 66  # V' chunk width (64 e + rowsum col + even pad)
NT = S // 128  # 8 t-chunks / s-chunks
F32 = mybir.dt.float32
F32R = mybir.dt.float32r


def build_nc():
    nc = bacc.Bacc(
        "TRN2",
        target_bir_lowering=False,
        debug=False,
        num_devices=1,
    )

    xt_d = nc.dram_tensor("xt", [H, 128, S], F32R, kind="ExternalInput").ap()
    wqk_d = nc.dram_tensor("wqk", [H, 128, 128], F32R, kind="ExternalInput").ap()
    wzq_d = nc.dram_tensor("wzq", [H, 128, 128], F32R, kind="ExternalInput").ap()
    wv_d = nc.dram_tensor("wv", [H, 128, VW], F32R, kind="ExternalInput").ap()
    ident_d = nc.dram_tensor("ident", [128, 128], F32R, kind="ExternalInput").ap()
    y_d = nc.dram_tensor("y", [S, D], F32, kind="ExternalOutput").ap()

    from contextlib import ExitStack

    with tile.TileContext(nc) as tc:
        with ExitStack() as ctx:
            _emit(ctx, tc, xt_d, wqk_d, wzq_d, wv_d, ident_d, y_d)

    nc.compile()
    return nc


def _emit(ctx, tc, xt_d, wqk_d, wzq_d, wv_d, ident_d, y_d):
    nc = tc.nc
    Exp = mybir.ActivationFunctionType.Exp

    consts = ctx.enter_context(tc.tile_pool(name="consts", bufs=1))
    qkt_pool = ctx.enter_context(tc.tile_pool(name="qkt", bufs=2))
    vp_pool = ctx.enter_context(tc.tile_pool(name="vp", bufs=2))
    attn_pool = ctx.enter_context(tc.tile_pool(name="attn", bufs=16))
    otsb_pool = ctx.enter_context(tc.tile_pool(name="otsb", bufs=2))
    recip_pool = ctx.enter_context(tc.tile_pool(name="recip", bufs=2))
    ps_sc = ctx.enter_context(tc.tile_pool(name="ps_sc", bufs=2, space="PSUM"))
    ps_ot = ctx.enter_context(tc.tile_pool(name="ps_ot", bufs=1, space="PSUM"))
    ps_misc = ctx.enter_context(tc.tile_pool(name="ps_misc", bufs=2, space="PSUM"))

    # ---- constant loads -------------------------------------------------
    wqk_sb = consts.tile([128, H, 128], F32R, name="wqk_sb")
    wzq_sb = consts.tile([128, H, 128], F32R, name="wzq_sb")
    wv_sb = consts.tile([128, H, VW], F32R, name="wv_sb")
    for sb, d in ((wqk_sb, wqk_d), (wzq_sb, wzq_d), (wv_sb, wv_d)):
        dt = d.rearrange("h p j -> p h j")
        nc.gpsimd.dma_start(out=sb[:, 0:1, :], in_=dt[:, 0:1, :])
        nc.gpsimd.dma_start(out=sb[:, 1:H, :], in_=dt[:, 1:H, :])
    # xt loads: heads 0-3 chunked on the SP sequencer (prologue critical
    # path); later heads as single DMAs issued from the idle GpSimd
    # sequencer so neither ACT nor DVE queues stall behind DMA issue.
    xt_sb = []
    for h in range(H):
        t = consts.tile([128, S], F32R, name=f"xt{h}")
        if h < 4:
            for c in range(4):
                nc.sync.dma_start(
                    out=t[:, 256 * c : 256 * c + 256],
                    in_=xt_d[h][:, 256 * c : 256 * c + 256],
                )
        else:
            nc.gpsimd.dma_start(out=t, in_=xt_d[h])
        xt_sb.append(t)

    ident = consts.tile([128, 128], F32R, name="ident")
    nc.gpsimd.dma_start(out=ident, in_=ident_d)

    ysb = consts.tile([128, NT, D], F32, name="ysb")

    # ---- software pipeline over heads ----------------------------------
    at_tiles = {}  # h -> list of 8 attnT sbuf tiles
    vp_sb = {}  # h -> V' sbuf tile [128, 8*66 + pad]
    ot_sb = {}  # h -> OT' sbuf tile [65, 1024]

    def proj_steps(h):
        """Yield small chunks of head-h projection work (Q/K/ZQ/V'), to be
        sprinkled between the exp-paced interleaved rounds."""
        # QK^T = wqk[h].T @ xt[h]  (rows 0-63 = Q^T, 64-127 = K^T)
        # ZQ   = wzq[h].T @ xt[h]  (rows 0-63 = 0,   64-127 = Q^T)
        qkt = qkt_pool.tile([128, S], F32R, tag="qkt")
        zq = qkt_pool.tile([128, S], F32R, tag="zq")
        proj_state[h] = (qkt, zq)
        for dst, w_sb in ((qkt, wqk_sb), (zq, wzq_sb)):
            for sh in range(2):
                # single-bank psum tiles from the misc pool: keeps the next
                # head's projection off the scores pool, whose slots recycle
                # at exp speed
                p_ps = ps_misc.tile([128, 512], F32, tag="misc")
                nc.tensor.matmul(
                    p_ps,
                    w_sb[:, h, :],
                    xt_sb[h][:, 512 * sh : 512 * sh + 512],
                    start=True,
                    stop=True,
                )
                nc.vector.tensor_copy(dst[:, 512 * sh : 512 * sh + 512], p_ps)
                yield

        # V' per t-chunk: [128, 66]; 4 chunks per 1-bank psum tile.
        # vp tail-padded so lhsT slices [66c : 66c+128] stay in-bounds.
        vp = vp_pool.tile([128, NT * VW + 64], F32R, tag="vp")
        vp_sb[h] = vp
        nc.gpsimd.memset(vp[:, NT * VW : NT * VW + 64].bitcast(F32), 0.0)
        for half in range(2):
            vp_ps = ps_misc.tile([128, 4 * VW], F32, tag="misc")
            for i in range(4):
                tcn = 4 * half + i
                nc.tensor.matmul(
                    vp_ps[:, VW * i : VW * i + VW],
                    xt_sb[h][:, 128 * tcn : 128 * tcn + 128],
                    wv_sb[:, h, :],
                    start=True,
                    stop=True,
                )
            nc.vector.tensor_copy(
                vp[:, 4 * VW * half : 4 * VW * half + 4 * VW], vp_ps
            )
            yield

    def emit_sc_av(h, hp, nxt=None):
        # Interleave this head's scores (whose matmuls stall on exp freeing
        # PSUM slots — exp is ~2x slower than a matmul pair) with the
        # previous head's attn@V accumulation so PE stays busy.
        # attn@V: OT'[e', s] = sum_t V'[t, e'] attnT[t, s], via lhsT = vp
        # 128-wide slice (M=128: cols 0-64 real, 65.. garbage), rhs = attnT.
        ats = []
        ot_ps = vp = None
        if hp is not None:
            ot_ps = ps_ot.tile([128, S], F32, tag="ot")
            vp = vp_sb[hp]
        for tcn in range(NT):
            if h is not None:
                qkt, zq = proj_state[h]
                sc_ps = ps_sc.tile([128, S], F32, tag="sc")
                lhsT = qkt[:, 128 * tcn : 128 * tcn + 128]  # [Q^T; K^T] chunk
                for sh in range(2):
                    nc.tensor.matmul(
                        sc_ps[:, 512 * sh : 512 * sh + 512],
                        lhsT,
                        zq[:, 512 * sh : 512 * sh + 512],
                        start=True,
                        stop=True,
                    )
                at = attn_pool.tile([128, S], F32R, tag="at")
                nc.scalar.activation(at, sc_ps, Exp)
                ats.append(at)
            if hp is not None:
                for sh in range(2):
                    # M=98: near-smallest col count spanning all four PE
                    # column groups (fp32r requirement) -> shorter LDWEIGHTS
                    nc.tensor.matmul(
                        ot_ps[0:98, 512 * sh : 512 * sh + 512],
                        vp[:, VW * tcn : VW * tcn + 98],
                        at_tiles[hp][tcn][:, 512 * sh : 512 * sh + 512],
                        start=(tcn == 0),
                        stop=(tcn == NT - 1),
                    )
            if nxt is not None:
                next(nxt, None)
        if h is not None:
            at_tiles[h] = ats
        if hp is not None:
            ot = otsb_pool.tile([98, S], F32R, tag="ot_sb")
            nc.vector.tensor_copy(ot, ot_ps[0:98, :])
            ot_sb[hp] = ot
            del at_tiles[hp]
            del vp_sb[hp]

    def emit_out(h):
        # PE-transpose OT' back to [s, e] in 128-chunks; col 64 = 8*rowsum
        ot = ot_sb[h]
        for half in range(2):
            ott_ps = ps_misc.tile([128, 4 * 128], F32R, tag="misc")
            for i in range(4):
                scn = 4 * half + i
                nc.tensor.transpose(
                    ott_ps[:, 128 * i : 128 * i + 98],
                    ot[:, 128 * scn : 128 * scn + 128],
                    ident[0:98, 0:98],
                )
            ottv = ott_ps.bitcast(F32).rearrange("p (c w) -> p c w", w=128)
            rec = recip_pool.tile([128, 4], F32, tag="rec")
            nc.vector.reciprocal(rec, ottv[:, :, 64])
            rec_b = bass.AP(
                tensor=rec.tensor, offset=rec.offset, ap=list(rec.ap) + [[0, 64]]
            )
            nc.vector.tensor_mul(
                ysb[:, 4 * half : 4 * half + 4, 64 * h : 64 * h + 64],
                ottv[:, :, 0:64],
                rec_b,
            )
        del ot_sb[h]

    proj_state = {}
    g = proj_steps(0)
    for _ in g:
        pass
    for h in range(H + 1):
        nxt = proj_steps(h + 1) if h + 1 < H else None
        emit_sc_av(h if h < H else None, h - 1 if h >= 1 else None, nxt)
        if nxt is not None:
            for _ in nxt:  # drain any remaining steps
                pass
        if h < H:
            proj_state.pop(h)
        if h >= 1:
            emit_out(h - 1)
        if h == 7:
            # flush the first half of the output columns (heads 0-5) while
            # the remaining heads compute, shrinking the final DMA tail
            for scn in range(NT):
                eng = nc.sync if scn % 2 == 0 else nc.gpsimd
                eng.dma_start(
                    out=y_d[128 * scn : 128 * scn + 128, 0:384],
                    in_=ysb[:, scn, 0:384],
                )

    # ---- store ----------------------------------------------------------
    for scn in range(NT):
        eng = nc.sync if scn % 2 == 0 else nc.gpsimd
        eng.dma_start(
            out=y_d[128 * scn : 128 * scn + 128, 384:D],
            in_=ysb[:, scn, 384:D],
        )


# --------------------------------------------------------------------------
# host side
# --------------------------------------------------------------------------

_NC_CACHE = {}

LAST_EXEC_NS = None
LAST_RESULTS = None


def _get_nc():
    if "nc" not in _NC_CACHE:
        _NC_CACHE["nc"] = build_nc()
    return _NC_CACHE["nc"]


def prep_inputs(x, Wq, bq, Wk, bk, Wv, bv):
    """Host-side layout prep. Returns per-core input maps."""
    x = np.ascontiguousarray(np.asarray(x, dtype=np.float32))
    Wq, bq = np.asarray(Wq, np.float32), np.asarray(bq, np.float32)
    Wk, bk = np.asarray(Wk, np.float32), np.asarray(bk, np.float32)
    Wv, bv = np.asarray(Wv, np.float32), np.asarray(bv, np.float32)

    # xt: [B, H, 128, S]: rows 0-63 = x^T, row 64 = ones, rows 65-127 = 0
    # (zero-padded to K=128 so every matmul keeps the full PE array active —
    #  half-height matmuls trip the HAM activity monitor into throttling)
    xt = np.zeros((B, H, 128, S), np.float32)
    xt[:, :, :HD] = x.transpose(0, 2, 1).reshape(B, H, HD, S)
    xt[:, :, HD] = 1.0

    def stack2(Wa, ba, Wb, bb):
        w = np.zeros((H, 128, 128), np.float32)
        w[:, :HD, :HD] = Wa.transpose(0, 2, 1)
        w[:, :HD, HD:] = Wb.transpose(0, 2, 1)
        w[:, HD, :HD] = ba
        w[:, HD, HD:] = bb
        return w

    wqk = stack2(Wq, bq, Wk, bk)
    wzq = np.zeros((H, 128, 128), np.float32)
    wzq[:, :HD, HD:] = Wq.transpose(0, 2, 1)
    wzq[:, HD, HD:] = bq

    wv = np.zeros((H, 128, VW), np.float32)
    wv[:, :HD, :HD] = Wv.transpose(0, 2, 1)
    wv[:, HD, :HD] = bv
    wv[:, HD, HD] = 8.0  # ones col scaled by sqrt(HD) -> folds post-softmax /8

    ident = np.eye(128, dtype=np.float32)

    return [
        {"xt": xt[b], "wqk": wqk, "wzq": wzq, "wv": wv, "ident": ident}
        for b in range(B)
    ]


def kernel(x, Wq, bq, Wk, bk, Wv, bv):
    global LAST_EXEC_NS, LAST_RESULTS
    from concourse.bass_utils import run_bass_kernel_spmd

    nc = _get_nc()
    in_maps = prep_inputs(x, Wq, bq, Wk, bk, Wv, bv)
    trace = os.environ.get("KERNEL_TRACE", "0") == "1"
    res = run_bass_kernel_spmd(
        nc,
        in_maps,
        core_ids=list(range(B)),
        trace=trace,
    )
    LAST_EXEC_NS = res.exec_time_ns
    LAST_RESULTS = res
    y = np.stack([res.results[b]["y"] for b in range(B)], axis=0)
    return y.astype(np.float32)


# revision 31
# speedup vs baseline: 1.0176x; 1.0176x over previous
"""Multi-head attention Bass/Tile kernel for Trainium2.

Problem: nn_MultiHeadAttention  (B=8, S=1024, D=768, H=12, HD=64)
  q = x_h @ Wq^T + bq ; k,v likewise (per head)
  scores = q @ k^T        (NO pre-softmax scaling)
  attn = softmax(scores, -1) / sqrt(64)
  out = attn @ v, heads concatenated -> [B, S, D]

Sharding: data parallel over batch, one sample per NeuronCore (8 cores).

All big matmuls run as float32r (1 cycle/row when N>=256, vs 4 for fp32).
fp32r ISA restrictions shape the layouts: every fp32r matmul needs M (out
partitions) spanning all 128 PE columns, even moving-dim counts, and
8-byte-aligned contiguous dst.

Per-core dataflow:
  xt[h]   : [65, 1024]  x_b^T per head + ones row (folds biases via K=65)
  wqk[h]  : [65, 128]   [[Wq^T | Wk^T], [bq | bk]]   -> QK^T   [128, 1024]
  wzq[h]  : [65, 128]   [[0 | Wq^T], [0 | bq]]       -> ZQ     [128, 1024]
      (scoresT = qkt_chunk.T @ zq: the zero rows of ZQ annihilate the
       Q-rows of the lhsT chunk, leaving K^T.T @ Q^T = scores transposed.
       K=128 keeps the whole PE array active so the HAM clock gate stays
       at full speed; K=64 matmuls read as half-idle and get throttled.)
  wv[h]   : [65, 66]    [[Wv^T, 0, 0], [bv, 8, 0]]   (66 = even-N pad;
       col 64 gives an 8*ones column -> attn row-sums * sqrt(64) for free)
  V'      : [128, 66] per t-chunk = x_chunk @ wv
  scoresT : [128, 1024] per t-chunk = K^T[:,tc].T @ Q^T
  attnT   : exp(scoresT)  (unnormalized; no max-subtraction needed in fp32)
  OT'     : [128, 1024]  = sum_tc  attnT[tc]^T.T @ V'[tc]... realized as
            lhsT=vp[66tc : 66tc+128] (M=128 incl. garbage cols), rhs=attnT
            -> rows 0-63 = out^T, row 64 = 8*rowsum, rows 65+ garbage
  OTT     : PE-transpose back to [s, e] chunks; col 64 = 8*rowsum
  y       : OTT[:, :64] * (1 / OTT[:, 64])
"""

import os
import sys

for _p in (
    "/opt/trn_rl_repo",
    "/root/.axon_site",
    "/root/.axon_site/_ro/trn_rl_repo",
    "/root/.axon_site/_ro/pypackages",
):
    if os.path.isdir(_p) and _p not in sys.path:
        sys.path.append(_p)

import numpy as np

import concourse.bacc as bacc
import concourse.bass as bass
import concourse.tile as tile
from concourse import mybir

B, S, D, H, HD = 8, 1024, 768, 12, 64
K1 = HD + 1  # 65: contraction dim with ones row for bias folding
VW = 66  # V' chunk width (64 e + rowsum col + even pad)
NT = S // 128  # 8 t-chunks / s-chunks
F32 = mybir.dt.float32
F32R = mybir.dt.float32r
BF16 = mybir.dt.bfloat16


def build_nc():
    nc = bacc.Bacc(
        "TRN2",
        target_bir_lowering=False,
        debug=False,
        num_devices=1,
    )

    xt_d = nc.dram_tensor("xt", [H, 128, S], F32R, kind="ExternalInput").ap()
    wqk_d = nc.dram_tensor("wqk", [H, 128, 128], F32R, kind="ExternalInput").ap()
    wzq_d = nc.dram_tensor("wzq", [H, 128, 128], F32R, kind="ExternalInput").ap()
    wv_d = nc.dram_tensor("wv", [H, 128, VW], F32R, kind="ExternalInput").ap()
    ident_d = nc.dram_tensor("ident", [128, 128], F32R, kind="ExternalInput").ap()
    y_d = nc.dram_tensor("y", [S, D], F32, kind="ExternalOutput").ap()

    from contextlib import ExitStack

    with tile.TileContext(nc) as tc:
        with ExitStack() as ctx:
            _emit(ctx, tc, xt_d, wqk_d, wzq_d, wv_d, ident_d, y_d)

    nc.compile()
    return nc


def _emit(ctx, tc, xt_d, wqk_d, wzq_d, wv_d, ident_d, y_d):
    nc = tc.nc
    Exp = mybir.ActivationFunctionType.Exp

    consts = ctx.enter_context(tc.tile_pool(name="consts", bufs=1))
    qkt_pool = ctx.enter_context(tc.tile_pool(name="qkt", bufs=2))
    vp_pool = ctx.enter_context(tc.tile_pool(name="vp", bufs=2))
    attn_pool = ctx.enter_context(tc.tile_pool(name="attn", bufs=16))
    otsb_pool = ctx.enter_context(tc.tile_pool(name="otsb", bufs=2))
    recip_pool = ctx.enter_context(tc.tile_pool(name="recip", bufs=2))
    ps_sc = ctx.enter_context(tc.tile_pool(name="ps_sc", bufs=2, space="PSUM"))
    ps_ot = ctx.enter_context(tc.tile_pool(name="ps_ot", bufs=1, space="PSUM"))
    ps_misc = ctx.enter_context(tc.tile_pool(name="ps_misc", bufs=2, space="PSUM"))

    # ---- constant loads -------------------------------------------------
    wqk_sb = consts.tile([128, H, 128], F32R, name="wqk_sb")
    wzq_sb = consts.tile([128, H, 128], F32R, name="wzq_sb")
    wv_sb = consts.tile([128, H, VW], F32R, name="wv_sb")
    for sb, d in ((wqk_sb, wqk_d), (wzq_sb, wzq_d), (wv_sb, wv_d)):
        dt = d.rearrange("h p j -> p h j")
        nc.gpsimd.dma_start(out=sb[:, 0:1, :], in_=dt[:, 0:1, :])
        nc.gpsimd.dma_start(out=sb[:, 1:H, :], in_=dt[:, 1:H, :])
    # xt loads: heads 0-3 chunked on the SP sequencer (prologue critical
    # path); later heads as single DMAs issued from the idle GpSimd
    # sequencer so neither ACT nor DVE queues stall behind DMA issue.
    xt_sb = []
    for h in range(H):
        t = consts.tile([128, S], F32R, name=f"xt{h}")
        if h < 4:
            for c in range(4):
                nc.sync.dma_start(
                    out=t[:, 256 * c : 256 * c + 256],
                    in_=xt_d[h][:, 256 * c : 256 * c + 256],
                )
        else:
            nc.gpsimd.dma_start(out=t, in_=xt_d[h])
        xt_sb.append(t)

    ident = consts.tile([128, 128], F32R, name="ident")
    nc.gpsimd.dma_start(out=ident, in_=ident_d)

    ysb = consts.tile([128, NT, D], F32, name="ysb")

    # ---- software pipeline over heads ----------------------------------
    at_tiles = {}  # h -> list of 8 attnT sbuf tiles
    vp_sb = {}  # h -> V' sbuf tile [128, 8*66 + pad]
    ot_sb = {}  # h -> OT' sbuf tile [65, 1024]

    def proj_steps(h):
        """Yield small chunks of head-h projection work (Q/K/ZQ/V'), to be
        sprinkled between the exp-paced interleaved rounds."""
        # QK^T = wqk[h].T @ xt[h]  (rows 0-63 = Q^T, 64-127 = K^T)
        # ZQ   = wzq[h].T @ xt[h]  (rows 0-63 = 0,   64-127 = Q^T)
        qkt = qkt_pool.tile([128, S], F32R, tag="qkt")
        zq = qkt_pool.tile([128, S], F32R, tag="zq")
        proj_state[h] = (qkt, zq)
        for dst, w_sb in ((qkt, wqk_sb), (zq, wzq_sb)):
            for sh in range(2):
                # single-bank psum tiles from the misc pool: keeps the next
                # head's projection off the scores pool, whose slots recycle
                # at exp speed
                p_ps = ps_misc.tile([128, 512], F32, tag="misc")
                nc.tensor.matmul(
                    p_ps,
                    w_sb[:, h, :],
                    xt_sb[h][:, 512 * sh : 512 * sh + 512],
                    start=True,
                    stop=True,
                )
                nc.vector.tensor_copy(dst[:, 512 * sh : 512 * sh + 512], p_ps)
                yield

        # V' per t-chunk: [128, 66]; 4 chunks per 1-bank psum tile.
        # vp tail-padded so lhsT slices [66c : 66c+128] stay in-bounds.
        vp = vp_pool.tile([128, NT * VW + 64], BF16, tag="vp")
        vp_sb[h] = vp
        nc.gpsimd.memset(vp[:, NT * VW : NT * VW + 64], 0.0)
        for half in range(2):
            vp_ps = ps_misc.tile([128, 4 * VW], F32, tag="misc")
            for i in range(4):
                tcn = 4 * half + i
                nc.tensor.matmul(
                    vp_ps[:, VW * i : VW * i + VW],
                    xt_sb[h][:, 128 * tcn : 128 * tcn + 128],
                    wv_sb[:, h, :],
                    start=True,
                    stop=True,
                )
            nc.vector.tensor_copy(
                vp[:, 4 * VW * half : 4 * VW * half + 4 * VW], vp_ps
            )
            yield

    def emit_sc_av(h, hp, nxt=None):
        # Interleave this head's scores (whose matmuls stall on exp freeing
        # PSUM slots — exp is ~2x slower than a matmul pair) with the
        # previous head's attn@V accumulation so PE stays busy.
        # attn@V: OT'[e', s] = sum_t V'[t, e'] attnT[t, s], via lhsT = vp
        # 128-wide slice (M=128: cols 0-64 real, 65.. garbage), rhs = attnT.
        ats = []
        ot_ps = vp = None
        if hp is not None:
            ot_ps = ps_ot.tile([128, S], F32, tag="ot")
            vp = vp_sb[hp]
        for tcn in range(NT):
            if h is not None:
                qkt, zq = proj_state[h]
                sc_ps = ps_sc.tile([128, S], F32, tag="sc")
                lhsT = qkt[:, 128 * tcn : 128 * tcn + 128]  # [Q^T; K^T] chunk
                for sh in range(2):
                    nc.tensor.matmul(
                        sc_ps[:, 512 * sh : 512 * sh + 512],
                        lhsT,
                        zq[:, 512 * sh : 512 * sh + 512],
                        start=True,
                        stop=True,
                    )
                at = attn_pool.tile([128, S], BF16, tag="at")
                nc.scalar.activation(at, sc_ps, Exp)
                ats.append(at)
            if hp is not None:
                for sh in range(2):
                    # M=98: near-smallest col count spanning all four PE
                    # column groups (fp32r requirement) -> shorter LDWEIGHTS
                    nc.tensor.matmul(
                        ot_ps[0:98, 512 * sh : 512 * sh + 512],
                        vp[:, VW * tcn : VW * tcn + 98],
                        at_tiles[hp][tcn][:, 512 * sh : 512 * sh + 512],
                        start=(tcn == 0),
                        stop=(tcn == NT - 1),
                    )
            if nxt is not None:
                next(nxt, None)
        if h is not None:
            at_tiles[h] = ats
        if hp is not None:
            ot = otsb_pool.tile([98, S], F32R, tag="ot_sb")
            nc.vector.tensor_copy(ot, ot_ps[0:98, :])
            ot_sb[hp] = ot
            del at_tiles[hp]
            del vp_sb[hp]

    def emit_out(h):
        # PE-transpose OT' back to [s, e] in 128-chunks; col 64 = 8*rowsum
        ot = ot_sb[h]
        for half in range(2):
            ott_ps = ps_misc.tile([128, 4 * 128], F32R, tag="misc")
            for i in range(4):
                scn = 4 * half + i
                nc.tensor.transpose(
                    ott_ps[:, 128 * i : 128 * i + 98],
                    ot[:, 128 * scn : 128 * scn + 128],
                    ident[0:98, 0:98],
                )
            ottv = ott_ps.bitcast(F32).rearrange("p (c w) -> p c w", w=128)
            rec = recip_pool.tile([128, 4], F32, tag="rec")
            nc.vector.reciprocal(rec, ottv[:, :, 64])
            rec_b = bass.AP(
                tensor=rec.tensor, offset=rec.offset, ap=list(rec.ap) + [[0, 64]]
            )
            nc.vector.tensor_mul(
                ysb[:, 4 * half : 4 * half + 4, 64 * h : 64 * h + 64],
                ottv[:, :, 0:64],
                rec_b,
            )
        del ot_sb[h]

    proj_state = {}
    g = proj_steps(0)
    for _ in g:
        pass
    for h in range(H + 1):
        nxt = proj_steps(h + 1) if h + 1 < H else None
        emit_sc_av(h if h < H else None, h - 1 if h >= 1 else None, nxt)
        if nxt is not None:
            for _ in nxt:  # drain any remaining steps
                pass
        if h < H:
            proj_state.pop(h)
        if h >= 1:
            emit_out(h - 1)
        if h == 7:
            # flush the first half of the output columns (heads 0-5) while
            # the remaining heads compute, shrinking the final DMA tail
            for scn in range(NT):
                eng = nc.sync if scn % 2 == 0 else nc.gpsimd
                eng.dma_start(
                    out=y_d[128 * scn : 128 * scn + 128, 0:384],
                    in_=ysb[:, scn, 0:384],
                )

    # ---- store ----------------------------------------------------------
    for scn in range(NT):
        eng = nc.sync if scn % 2 == 0 else nc.gpsimd
        eng.dma_start(
            out=y_d[128 * scn : 128 * scn + 128, 384:D],
            in_=ysb[:, scn, 384:D],
        )


# --------------------------------------------------------------------------
# host side
# --------------------------------------------------------------------------

_NC_CACHE = {}

LAST_EXEC_NS = None
LAST_RESULTS = None


def _get_nc():
    if "nc" not in _NC_CACHE:
        _NC_CACHE["nc"] = build_nc()
    return _NC_CACHE["nc"]


def prep_inputs(x, Wq, bq, Wk, bk, Wv, bv):
    """Host-side layout prep. Returns per-core input maps."""
    x = np.ascontiguousarray(np.asarray(x, dtype=np.float32))
    Wq, bq = np.asarray(Wq, np.float32), np.asarray(bq, np.float32)
    Wk, bk = np.asarray(Wk, np.float32), np.asarray(bk, np.float32)
    Wv, bv = np.asarray(Wv, np.float32), np.asarray(bv, np.float32)

    # xt: [B, H, 128, S]: rows 0-63 = x^T, row 64 = ones, rows 65-127 = 0
    # (zero-padded to K=128 so every matmul keeps the full PE array active —
    #  half-height matmuls trip the HAM activity monitor into throttling)
    xt = np.zeros((B, H, 128, S), np.float32)
    xt[:, :, :HD] = x.transpose(0, 2, 1).reshape(B, H, HD, S)
    xt[:, :, HD] = 1.0

    def stack2(Wa, ba, Wb, bb):
        w = np.zeros((H, 128, 128), np.float32)
        w[:, :HD, :HD] = Wa.transpose(0, 2, 1)
        w[:, :HD, HD:] = Wb.transpose(0, 2, 1)
        w[:, HD, :HD] = ba
        w[:, HD, HD:] = bb
        return w

    wqk = stack2(Wq, bq, Wk, bk)
    wzq = np.zeros((H, 128, 128), np.float32)
    wzq[:, :HD, HD:] = Wq.transpose(0, 2, 1)
    wzq[:, HD, HD:] = bq

    wv = np.zeros((H, 128, VW), np.float32)
    wv[:, :HD, :HD] = Wv.transpose(0, 2, 1)
    wv[:, HD, :HD] = bv
    wv[:, HD, HD] = 8.0  # ones col scaled by sqrt(HD) -> folds post-softmax /8

    ident = np.eye(128, dtype=np.float32)

    return [
        {"xt": xt[b], "wqk": wqk, "wzq": wzq, "wv": wv, "ident": ident}
        for b in range(B)
    ]


def kernel(x, Wq, bq, Wk, bk, Wv, bv):
    global LAST_EXEC_NS, LAST_RESULTS
    from concourse.bass_utils import run_bass_kernel_spmd

    nc = _get_nc()
    in_maps = prep_inputs(x, Wq, bq, Wk, bk, Wv, bv)
    trace = os.environ.get("KERNEL_TRACE", "0") == "1"
    res = run_bass_kernel_spmd(
        nc,
        in_maps,
        core_ids=list(range(B)),
        trace=trace,
    )
    LAST_EXEC_NS = res.exec_time_ns
    LAST_RESULTS = res
    y = np.stack([res.results[b]["y"] for b in range(B)], axis=0)
    return y.astype(np.float32)
